# revision 30
# baseline (speedup 1.0000x reference)
"""BinaryMLP (dense_mlp) Trainium2 kernel — 8-core data-parallel sync-BN.

Strategy:
  - Shard batch (4096) across 8 NeuronCores (512 rows each); replicate weights.
  - Activations live in SBUF transposed: [features -> partitions, batch -> free].
    BatchNorm batch stats are then free-axis reductions (DVE / ACT accum).
  - Matmuls: lhsT = W.T tile (stationary), rhs = xT tile (moving), fp32 PSUM
    accumulation. The PE is power-throttled to K=13/16 (1.95 GHz) in steady
    state, so each 512-row matmul issues every ~263 ns with LDWEIGHTS fully
    hidden — the kernel is instruction-count-bound on the PE.
  - Layer 0 runs a 12-k-tile bf16 head plus a 20-k-tile fp8e4 DoubleRow tail
    (x/4 vs 4*W0 — the power-of-2 scales cancel exactly in the product);
    layers 1/2 run fully fp8 DoubleRow (sign() weights are exactly +-1 in
    fp8). h1/h2 are stored offset-coded (h - 0.40625) to center the
    post-ReLU distribution; the constant per-feature shift downstream is
    absorbed exactly by the next BatchNorm's mean subtraction.
  - L0's k-loop runs the fp8 chunks FIRST: the fp8 x tail (1.3 MB) loads in
    a few us so the PE starts ~11 us in, while the bigger bf16 x head
    streams in behind it.
  - Sync-BN: per-group pipelined stats with BATCHED AllReduces (2 groups =
    8 KB per AR). Each AR pays a ~10 us ncfw latency floor on the serial CC
    engine, so fewer, earlier-triggered ARs make the last apply land ~15 us
    after a layer's last matmul instead of ~50 us. Pending applies drain
    BEFORE the next group's stats in the in-order ACT/DVE queues, so an
    in-flight AR never delays the stats that free PSUM banks for the PE.
    Each consumer layer defers the k-chunks that read the producer's last
    two groups (kb) behind ~40 us of covered pass-A work.
  - Final Linear flips layout back to [batch -> partitions, classes -> free]
    by using the h3 activation tiles as the stationary operand; log_softmax
    is a free-axis max/exp-accum/ln chain; b3 is folded in via an extra
    ones-row x b3-row contraction tile. The Exp/Ln ACT tables are re-warmed
    right after the last BN apply (whose Sqrt evicts them) so no 1.3 us
    table load lands on the softmax critical path.
"""

import os
import sys

for _p in ("/opt/trn_rl_repo",):
    if _p not in sys.path and os.path.isdir(_p):
        sys.path.insert(0, _p)

import numpy as np
import ml_dtypes

import concourse.bass as bass
import concourse.mybir as mybir
import concourse.tile as tile
from concourse import bacc
from concourse.bass_utils import run_bass_kernel_spmd

AF = mybir.ActivationFunctionType
ALU = mybir.AluOpType
F32 = mybir.dt.float32
BF16 = mybir.dt.bfloat16
F8 = mybir.dt.float8e4
AX = mybir.AxisListType
PM = mybir.MatmulPerfMode

NP_BF16 = ml_dtypes.bfloat16
NP_F8 = ml_dtypes.float8_e4m3

P = 128
N_CORES = 8
B_TOTAL = 4096
D_IN = 4096
H1, H2, H3 = 4096, 4096, 2048
C = 1000
BN_EPS = 1e-5

KT0, MT0 = D_IN // P, H1 // P  # 32, 32
KT1, MT1 = H1 // P, H2 // P  # 32, 32
KT2, MT2 = H2 // P, H3 // P  # 32, 16
KT3 = H3 // P  # 16 (+1 aug tile for the bias)
MG = 4  # out-feature tiles per PSUM group (4 banks; 2 groups in flight)
KPAIR = 4  # k-tiles per weight-slab DMA (512KB slabs amortize DMA fixed cost)
# fp8 activation offset: h1/h2 are stored as (relu(bn(h)) - OFFC) in e4m3,
# centering the post-ReLU distribution to cut quantization noise ~30%. The
# resulting constant per-feature shift downstream is absorbed by the next
# BatchNorm's mean subtraction. 13/32 is exact in both bf16 and e4m3.
OFFC = 0.40625
# Layer 0 runs k-tiles KF8_START..31 in fp8 DoubleRow: x/XDIV in e4m3 against
# XDIV*W0 in e4m3 — the power-of-2 scales cancel exactly in the product, so
# the fp8 partial sums accumulate into the same PSUM as the bf16 head with no
# correction. XDIV=4 keeps small |x| out of the e4m3 denormal floor; numpy
# simulation of the full pipeline puts absmax-rel at 1.67e-2 (< 2e-2 gate).
KF8_START = 12
XDIV = 4.0


def build(b_shard: int, n_cores: int):
    """Build + compile the SPMD program for a per-core batch shard of b_shard."""
    assert b_shard % P == 0
    nb = b_shard // P  # batch tiles for the final layer
    n_batch_global = b_shard * n_cores
    inv_n = 1.0 / float(n_batch_global)
    rg = [list(range(n_cores))]

    nc = bacc.Bacc(
        "TRN2", target_bir_lowering=False, debug=False, num_devices=n_cores
    )

    xT = nc.dram_tensor(
        "xT", [KF8_START * P, b_shard], BF16, kind="ExternalInput"
    ).ap()
    xT8 = nc.dram_tensor(
        "xT8", [(KT0 - KF8_START) * P, b_shard], F8, kind="ExternalInput"
    ).ap()
    w0t = nc.dram_tensor(
        "w0t", [KF8_START * P, H1], BF16, kind="ExternalInput"
    ).ap()
    w0t8 = nc.dram_tensor(
        "w0t8", [(KT0 - KF8_START) * P, H1], F8, kind="ExternalInput"
    ).ap()
    w1t = nc.dram_tensor("w1t", [H1, H2], F8, kind="ExternalInput").ap()
    w2t = nc.dram_tensor("w2t", [H2, H3], F8, kind="ExternalInput").ap()
    # W3.T augmented with a b3 row (row H3) + zero padding to a full k-tile.
    w3t = nc.dram_tensor("w3t", [(KT3 + 1) * P, C], BF16, kind="ExternalInput").ap()
    g0p = nc.dram_tensor("g0p", [P, MT0], F32, kind="ExternalInput").ap()
    b0p = nc.dram_tensor("b0p", [P, MT0], F32, kind="ExternalInput").ap()
    g1p = nc.dram_tensor("g1p", [P, MT1], F32, kind="ExternalInput").ap()
    b1p = nc.dram_tensor("b1p", [P, MT1], F32, kind="ExternalInput").ap()
    g2p = nc.dram_tensor("g2p", [P, MT2], F32, kind="ExternalInput").ap()
    b2p = nc.dram_tensor("b2p", [P, MT2], F32, kind="ExternalInput").ap()
    out = nc.dram_tensor("out", [b_shard, C], F32, kind="ExternalOutput").ap()

    with tile.TileContext(nc) as tc:
        with (
            tc.tile_pool(name="big", bufs=1) as big,
            tc.tile_pool(name="wpool", bufs=8) as wpool,
            tc.tile_pool(name="psum", bufs=8, space="PSUM") as psum,
            tc.tile_pool(name="scratch", bufs=4) as scratch,
            tc.tile_pool(name="outp", bufs=8) as outp,
            tc.tile_pool(name="bn", bufs=10) as bnp,
            tc.tile_pool(name="small", bufs=24) as small,
            tc.tile_pool(name="dram", bufs=1, space="DRAM") as dram,
        ):
            # ---- persistent activation buffers -------------------------------
            # xT_sb holds only the bf16 head (k-tiles 0..KF8_START-1); the
            # fp8 tail lives in xT8_sb.
            xT_sb = big.tile(
                [P, KF8_START, b_shard], BF16, name="xT_sb", tag="xT_sb"
            )
            xT8_sb = big.tile(
                [P, KT0 - KF8_START, b_shard], F8, name="xT8_sb", tag="xT8_sb"
            )
            h1_sb = big.tile([P, MT0, b_shard], F8, name="h1_sb", tag="h1_sb")
            h2_sb = big.tile([P, MT1, b_shard], F8, name="h2_sb", tag="h2_sb")
            h3_sb = big.tile([P, MT2, b_shard], BF16, name="h3_sb", tag="h3_sb")
            # one shared pre-BN scratch: layer N+1's first stats-write lands
            # only after its k-loop consumed ALL of layer N's output, i.e.
            # after every layer-N apply (the last pre readers) completed
            pre0 = big.tile([P, MT0, b_shard], BF16, name="pre0", tag="pre0")
            # dedicated stash buffer for spilled pass-A partials: carried-over
            # applies from the previous layer still read pre0's old tiles, so
            # the stash must not alias pre0. Two slots: L1 stashes its last
            # TWO groups' pass-A partials to extend the covered window ahead
            # of the cross-core join by another ~8us.
            stash_sb = big.tile(
                [P, 2 * MG, b_shard], BF16, name="stash_sb", tag="stash_sb"
            )
            ones_t = big.tile([P, b_shard], BF16, name="ones_t", tag="ones_t")
            w3_sb = big.tile([P, KT3 + 1, C], BF16, name="w3_sb", tag="w3_sb")

            # BN gamma/beta tiles (host packed to [P, MT]); DMAs for g0/b0 are
            # emitted from the x loader so the startup gpsimd queue stays
            # clear for the first fp8 x chunks.
            gb = {}

            def alloc_gb(specs):
                for nm, _ap, mt in specs:
                    gb[nm] = big.tile(
                        [P, mt], F32, name=f"{nm}_sb", tag=f"{nm}_sb"
                    )

            def emit_gb(specs):
                for nm, ap_, _mt in specs:
                    nc.gpsimd.dma_start(gb[nm][:], ap_)

            gb0_specs = (("g0", g0p, MT0), ("b0", b0p, MT0))
            alloc_gb(gb0_specs)

            # ---- L0 input loader --------------------------------------------
            # L0's k-chunks are emitted fp8-FIRST (order below): the fp8 x
            # tail chunks are small (256KB) and load on gpsimd within a few
            # us, so the PE starts ~11us in; the bf16 head chunks stream on
            # the sync queue (interleaved with the weight slabs) during the
            # ~21us of fp8-phase compute. w3 (4.25MB) and the g0/b0 BN params
            # are deferred into the bf16 phase so they never race the
            # startup-critical pieces.
            L0_ORDER = [12, 16, 20, 24, 28, 0, 4, 8]
            assert L0_ORDER[0] == KF8_START
            l0_pos = {kp: i for i, kp in enumerate(L0_ORDER)}
            xT_r = xT.rearrange("(ko p) b -> p ko b", p=P)
            xT8_r = xT8.rearrange("(ko p) b -> p ko b", p=P)
            piece_emitted = {kp: False for kp in L0_ORDER}
            gb0_emitted = [False]

            # fp8 x chunks alternate between the gpsimd and scalar queues
            # (both idle at startup; only gpsimd/sync/scalar can issue DMAs)
            # so the early transfers spread across more DMA rings in
            # parallel while the rings are still ramping up.
            x8_queues = [nc.gpsimd, nc.scalar]

            N_F8C = (KT0 - KF8_START) // KPAIR  # fp8 chunk count (first in order)

            def xT_loader(kp):
                pos = l0_pos[kp]
                # all fp8 pieces issue immediately (they are what the PE
                # consumes first; two queues drain them in parallel), bf16
                # pieces follow with a 2-chunk lookahead
                for p_ in range(max(min(pos + 2, len(L0_ORDER) - 1), N_F8C - 1) + 1):
                    c = L0_ORDER[p_]
                    if piece_emitted[c]:
                        continue
                    piece_emitted[c] = True
                    if c >= KF8_START:
                        c8 = c - KF8_START
                        if c8 == 0:
                            # the very first piece gates the first matmul:
                            # split it across both queues so two DMA rings
                            # pull it in parallel while they're still cold
                            h8 = KPAIR // 2
                            nc.gpsimd.dma_start(
                                xT8_sb[:, :h8, :], xT8_r[:, :h8, :]
                            )
                            nc.scalar.dma_start(
                                xT8_sb[:, h8:KPAIR, :], xT8_r[:, h8:KPAIR, :]
                            )
                        else:
                            eng = x8_queues[(c8 // KPAIR) % len(x8_queues)]
                            eng.dma_start(
                                xT8_sb[:, c8 : c8 + KPAIR, :],
                                xT8_r[:, c8 : c8 + KPAIR, :],
                            )
                    else:
                        nc.sync.dma_start(
                            xT_sb[:, c : c + KPAIR, :],
                            xT_r[:, c : c + KPAIR, :],
                        )
                if kp == 0 and not gb0_emitted[0]:
                    gb0_emitted[0] = True
                    emit_gb(gb0_specs)

            def mlp_layer(
                lidx, in_sb, kt, mt, w_dram, g_sb, b_sb, out_sb, pre_sb,
                ka=None, kb=(), in_loader=None, fp8=False, out_off=None,
                stash_groups=0, ar_batches=None, fp8_tail=None, lag=1,
                carry_in=None,
            ):
                """out_sb <- relu(bn(in_sb.T @ W.T)), transposed layout.

                Per-group pipelined sync-BN: every PSUM group's stats are
                all-reduced (batched with its AR partners) and applied after
                a short lag. ka/kb: KPAIR-aligned k-chunk lists. Pass A (ka)
                runs first for the leading groups and only touches input
                tiles whose producer applies completed early; pass B (kb)
                consumes the late tiles. stash_groups leading groups spill
                pass-A partials to the stash buffer to stay within the
                8-bank PSUM limit while extending the covered window.
                """
                ngroups = mt // MG
                S = stash_groups
                assert S <= 2, "stash_sb holds two groups' partials"
                stash_slot = {}
                if ka is None:
                    ka = list(range(0, kt, KPAIR))
                ka = list(ka)
                kb = list(kb)
                kstep = 2 if fp8 else 1
                k_first = ka[0]
                last_kp = kb[-1] if kb else ka[-1]
                last_fp8 = fp8 or (fp8_tail is not None and last_kp >= fp8_tail[0])
                k_last = last_kp + KPAIR - (2 if last_fp8 else 1)

                ps_tiles = {}
                # Stats AllReduces are BATCHED (2 groups = 8KB payload per
                # AR): the CC engine services collectives serially with a
                # ~10us ncfw latency floor each, so halving the AR count
                # makes the last apply land ~15us after the layer's last
                # matmul instead of ~50us.
                if ar_batches is None:
                    ar_batches = [[g] for g in range(ngroups)]
                bat_of = {}
                for bi, bat in enumerate(ar_batches):
                    for ii, g in enumerate(bat):
                        bat_of[g] = (bi, ii)
                stats_b = [
                    big.tile(
                        [P, 2 * MG * len(bat)], F32,
                        name=f"st{lidx}_{bi}", tag=f"st{lidx}_{bi}",
                    )
                    for bi, bat in enumerate(ar_batches)
                ]
                gstats_b = [
                    big.tile(
                        [P, 2 * MG * len(bat)], F32,
                        name=f"gst{lidx}_{bi}", tag=f"gst{lidx}_{bi}",
                    )
                    for bi, bat in enumerate(ar_batches)
                ]
                arin_b = [
                    dram.tile(
                        [P, 2 * MG * len(bat)], F32,
                        name=f"ari{lidx}_{bi}", tag=f"ari{lidx}_{bi}",
                    )
                    for bi, bat in enumerate(ar_batches)
                ]
                arout_b = [
                    dram.tile(
                        [P, 2 * MG * len(bat)], F32,
                        name=f"aro{lidx}_{bi}", tag=f"aro{lidx}_{bi}",
                    )
                    for bi, bat in enumerate(ar_batches)
                ]

                def emit_mms(g, kps, resume=False, stop_k=None):
                    if g not in ps_tiles:
                        ps_tiles[g] = [
                            psum.tile(
                                [P, b_shard], F32,
                                name=f"ps{lidx}_{g}_{kps[0]}_{j}", tag="ps",
                            )
                            for j in range(MG)
                        ]
                    ps = ps_tiles[g]
                    ks = stop_k if stop_k is not None else k_last
                    for kp in kps:
                        # fp8_tail: (start_k, in8_sb, w8_dram) — chunks at or
                        # past start_k run fp8 DoubleRow from the scaled-fp8
                        # copies (x/XDIV fp8 vs XDIV*W0 fp8: the scales cancel
                        # exactly in the product, so PSUM accumulation matches
                        # the bf16 part with no epilogue correction).
                        c_fp8 = fp8 or (fp8_tail is not None and kp >= fp8_tail[0])
                        if fp8_tail is not None and kp >= fp8_tail[0]:
                            k0t, c_in, c_w = fp8_tail
                            w_src = c_w[
                                (kp - k0t) * P : (kp - k0t + KPAIR) * P,
                                g * MG * P : (g + 1) * MG * P,
                            ]
                        else:
                            k0t, c_in, c_w = 0, in_sb, None
                            w_src = w_dram[
                                kp * P : (kp + KPAIR) * P,
                                g * MG * P : (g + 1) * MG * P,
                            ]
                        if in_loader is not None:
                            in_loader(kp)
                        slab = wpool.tile(
                            [P, KPAIR, MG * P], F8 if c_fp8 else BF16,
                            name=f"w{lidx}_{g}_{kp}", tag="wslab",
                        )
                        # during L0's ramp phase (DMA rings still cold) g1's
                        # first slabs issue from the scalar queue so the two
                        # interleaved groups' weight streams pull in parallel
                        seng = nc.sync
                        if in_loader is not None and g == 1 and l0_pos[kp] < 3:
                            seng = nc.scalar
                        seng.dma_start(
                            slab[:], w_src.rearrange("(kk p) c -> p kk c", p=P)
                        )
                        for kk in range(0, KPAIR, 2 if c_fp8 else 1):
                            k = kp + kk
                            for j in range(MG):
                                if c_fp8:
                                    nc.tensor.matmul(
                                        ps[j][:],
                                        slab[:, kk : kk + 2, j * P : (j + 1) * P],
                                        c_in[:, k - k0t : k - k0t + 2, :],
                                        start=(k == k_first and not resume),
                                        stop=(k == ks),
                                        perf_mode=PM.DoubleRow,
                                        skip_group_check=resume,
                                    )
                                else:
                                    nc.tensor.matmul(
                                        ps[j][:],
                                        slab[:, kk, j * P : (j + 1) * P],
                                        in_sb[:, k, :],
                                        start=(k == k_first and not resume),
                                        stop=(k == ks),
                                        skip_group_check=resume,
                                    )

                def emit_stash(g, slot):
                    # spill pass-A partials to the stash buffer (bf16) and
                    # release the PSUM banks for more covered-pass work
                    stash_slot[g] = slot
                    for j in range(MG):
                        nc.vector.tensor_scalar(
                            stash_sb[:, slot * MG + j, :], ps_tiles[g][j][:],
                            1.0, None, ALU.mult,
                        )
                    del ps_tiles[g]

                def emit_unstash(g):
                    # reload stashed partials into fresh PSUM banks; resumed
                    # matmuls accumulate on top with start=False
                    slot = stash_slot[g]
                    ps_tiles[g] = [
                        psum.tile(
                            [P, b_shard], F32, name=f"psr{lidx}_{g}_{j}", tag="ps"
                        )
                        for j in range(MG)
                    ]
                    for j in range(MG):
                        nc.scalar.activation(
                            ps_tiles[g][j][:], stash_sb[:, slot * MG + j, :],
                            AF.Copy,
                        )

                def emit_stats(g):
                    bi, ii = bat_of[g]
                    st = stats_b[bi]
                    off = 2 * MG * ii
                    for j in range(MG):
                        m = g * MG + j
                        # DVE: PSUM -> bf16 pre-BN copy, fused with the
                        # per-feature sum via accum_out (one PSUM read)
                        nc.vector.tensor_scalar(
                            pre_sb[:, m, :],
                            ps_tiles[g][j][:],
                            1.0,
                            None,
                            ALU.mult,
                            ALU.add,
                            accum_out=st[:, off + j : off + j + 1],
                        )
                        sq = scratch.tile(
                            [P, b_shard], F32, name=f"sq{lidx}_{m}", tag="sq"
                        )
                        # HW: only one PSUM read per DVE inst, so square on ACT
                        nc.scalar.activation(
                            sq[:],
                            ps_tiles[g][j][:],
                            AF.Square,
                            accum_out=st[:, off + MG + j : off + MG + j + 1],
                        )

                def emit_ar(bi):
                    nc.gpsimd.dma_start(arin_b[bi][:], stats_b[bi][:])
                    nc.gpsimd.collective_compute(
                        "AllReduce",
                        ALU.add,
                        replica_groups=rg,
                        ins=[arin_b[bi].opt()],
                        outs=[arout_b[bi].opt()],
                    )
                    nc.gpsimd.dma_start(gstats_b[bi][:], arout_b[bi][:])

                def emit_apply(g):
                    # s = gamma * rsqrt(var+eps); t = beta - mean*s, then
                    # relu(h*s + t) per tile (ACT), with the fp8 offset
                    # subtract on DVE when out_off is set.
                    bi, ii = bat_of[g]
                    gs = gstats_b[bi][:, 2 * MG * ii : 2 * MG * (ii + 1)]
                    m0 = g * MG
                    mex = bnp.tile([P, 2 * MG], F32, name=f"mex{lidx}_{g}", tag="bn2")
                    m2 = bnp.tile([P, MG], F32, name=f"m2{lidx}_{g}", tag="bn")
                    var = bnp.tile([P, MG], F32, name=f"var{lidx}_{g}", tag="bn")
                    inv = bnp.tile([P, MG], F32, name=f"inv{lidx}_{g}", tag="bn")
                    rstd = bnp.tile([P, MG], F32, name=f"rstd{lidx}_{g}", tag="bn")
                    s_sb = bnp.tile([P, MG], F32, name=f"s{lidx}_{g}", tag="bn")
                    t_sb = bnp.tile([P, MG], F32, name=f"t{lidx}_{g}", tag="bn")
                    tmp = bnp.tile([P, MG], F32, name=f"tmp{lidx}_{g}", tag="bn")
                    nc.scalar.activation(mex[:], gs[:], AF.Copy, scale=inv_n)
                    mean = mex[:, :MG]
                    ex2 = mex[:, MG:]
                    nc.vector.tensor_mul(m2[:], mean[:], mean[:])
                    nc.vector.tensor_sub(var[:], ex2[:], m2[:])
                    nc.vector.tensor_scalar_add(var[:], var[:], BN_EPS)
                    nc.vector.reciprocal(inv[:], var[:])
                    nc.scalar.activation(rstd[:], inv[:], AF.Sqrt)
                    nc.vector.tensor_mul(s_sb[:], rstd[:], g_sb[:, m0 : m0 + MG])
                    nc.vector.tensor_mul(tmp[:], mean[:], s_sb[:])
                    nc.vector.tensor_sub(t_sb[:], b_sb[:, m0 : m0 + MG], tmp[:])
                    for j in range(MG):
                        m = m0 + j
                        if out_off is None:
                            nc.scalar.activation(
                                out_sb[:, m, :],
                                pre_sb[:, m, :],
                                AF.Relu,
                                bias=t_sb[:, j : j + 1],
                                scale=s_sb[:, j : j + 1],
                            )
                        else:
                            nc.scalar.activation(
                                pre_sb[:, m, :],
                                pre_sb[:, m, :],
                                AF.Relu,
                                bias=t_sb[:, j : j + 1],
                                scale=s_sb[:, j : j + 1],
                            )
                            nc.vector.tensor_scalar(
                                out_sb[:, m, :],
                                pre_sb[:, m, :],
                                out_off,
                                None,
                                ALU.subtract,
                            )

                # Applies drain LAGGED by `lag` batches, and BEFORE the
                # just-finished group's stats: the drained applies' ARs are
                # a full batch cadence old (long complete), and putting them
                # ahead of the stats in the in-order ACT/DVE queues lets
                # them execute as soon as their AR lands instead of queueing
                # behind the next group's PSUM reads.
                pending = []

                def finish(g):
                    bi, ii = bat_of[g]
                    last_in_batch = ii == len(ar_batches[bi]) - 1
                    if last_in_batch:
                        while len(pending) > lag:
                            emit_apply(pending.pop(0))
                    emit_stats(g)
                    if last_in_batch:
                        emit_ar(bi)
                        pending.extend(ar_batches[bi])

                # ---- emission schedule ----------------------------------
                # Finishes run in TILE ORDER (g0, g1, ..., g_last) so the next
                # layer can consume low tiles first and defer only the last
                # groups' tiles (kb). The LAST S groups run pass A up front
                # (stash to stash_sb, freeing PSUM) to extend the covered
                # window; their unstashes hide under the last full groups'
                # matmul time and they resume at the very end.
                stashed = list(range(ngroups - S, ngroups))
                fulls = list(range(2, ngroups - S))
                if in_loader is not None:
                    # input layer: interleave the two open groups in 4-k
                    # sub-blocks so the PE starts on the first x chunk
                    assert S == 0
                    for kp in ka:
                        emit_mms(0, [kp])
                        emit_mms(1, [kp])
                else:
                    for si, g in enumerate(stashed):
                        # pass A closed with stop=True; the resume reloads the
                        # spilled partials and accumulates pass B on top
                        emit_mms(g, ka, stop_k=ka[-1] + KPAIR - kstep)
                        emit_stash(g, si)
                    emit_mms(0, ka)
                    emit_mms(1, ka)
                # the previous layer's leftover (AR-gated) applies are emitted
                # HERE — behind this layer's covered pass-A matmuls, but ahead
                # of its PE-critical stats ops, so a pending AR never
                # back-pressures the PE through the in-order DVE/ACT queues
                if carry_in:
                    for ap_fn in carry_in:
                        ap_fn()
                for g in (0, 1):
                    if kb:
                        emit_mms(g, kb)
                    finish(g)
                # the first stashed group's unstash hides under the last full
                # group's matmuls; later stashed groups unstash just before
                # the PREVIOUS one's resume (their copies overlap its ~8us of
                # kb matmuls), keeping peak PSUM at 8 banks throughout
                for i, g in enumerate(fulls):
                    emit_mms(g, ka + kb)
                    if i == len(fulls) - 1 and S >= 1:
                        emit_unstash(stashed[0])
                    finish(g)
                for si, g in enumerate(stashed):
                    if si + 1 < len(stashed):
                        emit_unstash(stashed[si + 1])
                    emit_mms(g, kb, resume=True)
                    finish(g)
                # leftover applies are NOT drained here: their ARs may still
                # be in flight, and draining would queue them ahead of the
                # next layer's stats. The caller passes them to the next
                # layer's carry_in (or the L3 epilogue).
                return [
                    (lambda gg=g: emit_apply(gg)) for g in pending
                ]

            # lag=2 for L0: before any cross-core join the cores are skewed
            # by up to ~50us (launch stagger varies run to run), so L0's ARs
            # can take 35-50us — a two-batch (~90us) drain margin keeps a
            # slow AR's apply from blocking later groups' stats (which free
            # PSUM banks for the PE) in the in-order ACT/DVE queues.
            carry0 = mlp_layer(
                0, xT_sb, KT0, MT0, w0t, gb["g0"], gb["b0"], h1_sb, pre0,
                ka=L0_ORDER, in_loader=xT_loader, out_off=OFFC,
                fp8_tail=(KF8_START, xT8_sb, w0t8), lag=2,
                ar_batches=[[0, 1], [2, 3], [4, 5], [6, 7]],
            )
            # ones_t (L3 bias row) is initialized here so its memsets sit
            # behind the startup-critical x-chunk DMAs on the gpsimd queue
            nc.gpsimd.memset(ones_t[:], 0.0)
            nc.gpsimd.memset(ones_t[:1, :], 1.0)
            gb12_specs = (
                ("g1", g1p, MT1),
                ("b1", b1p, MT1),
                ("g2", g2p, MT2),
                ("b2", b2p, MT2),
            )
            alloc_gb(gb12_specs)
            emit_gb(gb12_specs)
            # w3 (4.25MB) is issued HERE — on the gpsimd queue this sits
            # behind L0's gstats returns, so the transfer starts only once
            # L0's last AR lands (~250us), far from the startup-critical
            # x-chunk/slab window (issuing it at t~14us steals ring
            # bandwidth and stalls the PE's early feed) yet well before
            # L3 reads it (~430us).
            nc.gpsimd.dma_start(
                w3_sb[:], w3t.rearrange("(ko p) c -> p ko c", p=P)
            )
            # carry0=[4,5,6,7] (h1 tiles 16..31): L1's pass A may only read
            # tiles 0..15, whose applies were emitted inside L0. The kb
            # chunks are ordered ascending so the single cross-core JOIN
            # (waiting out the launch skew via L0's last AR) lands as late
            # as possible behind covered work, and is paid exactly once —
            # every later AR runs on skew-converged cores at ~10-15us.
            carry1 = mlp_layer(
                1, h1_sb, KT1, MT1, w1t, gb["g1"], gb["b1"], h2_sb, pre0,
                ka=list(range(0, 16, KPAIR)), kb=[16, 20, 24, 28], fp8=True,
                out_off=OFFC, stash_groups=2, lag=1,
                ar_batches=[[0, 1], [2, 3], [4, 5], [6, 7]],
                carry_in=carry0,
            )

            # carry2=[1,2,3] (h3 tiles 4..15): L3 phase A0 (bias + tiles
            # 0..3) is the only work whose applies were emitted inside L2.
            # Two ARs of two groups each: a finer split serializes extra
            # ~15us ARs on the CC engine past L2's end and stalls L3.
            carry2 = mlp_layer(
                2, h2_sb, KT2, MT2, w2t, gb["g2"], gb["b2"], h3_sb, pre0,
                ka=list(range(0, 20, KPAIR)), kb=[20, 24, 28], fp8=True,
                stash_groups=1, lag=1, ar_batches=[[0, 1], [2, 3]],
                carry_in=carry1,
            )

            # ---- final Linear + log_softmax ---------------------------------
            # lhsT = h3 tile slice (stationary), rhs = preloaded W3.T slab
            # (moving). Output flips to [batch -> partitions, classes -> free].
            # Phase A0: k-OUTER (all 4 batch tiles per k) over [bias, 0..7] —
            # covered work while L2's carried applies land. Phase A1 consumes
            # tiles 8..11; phase B runs per-batch-tile over the last 4 k with
            # stop, so each tile's softmax chain starts while the next tile's
            # matmuls run instead of the whole softmax serializing after the
            # last matmul.
            half = (C + 1) // 2  # 500
            L3A0 = [KT3] + list(range(0, 4))
            L3A1 = list(range(4, 12))
            L3B = list(range(12, KT3))
            ps3 = [
                [
                    psum.tile([P, 512], F32, name=f"ps3_{b}_{h}", tag="ps")
                    for h in range(2)
                ]
                for b in range(nb)
            ]

            def l3_mms(b, ks):
                for k in ks:
                    lhsT = (
                        h3_sb[:, k, b * P : (b + 1) * P]
                        if k < KT3
                        else ones_t[:, b * P : (b + 1) * P]
                    )
                    for h in range(2):
                        nc.tensor.matmul(
                            ps3[b][h][:, : half],
                            lhsT,
                            w3_sb[:, k, h * half : (h + 1) * half],
                            start=(k == KT3),
                            stop=(k == L3B[-1]),
                        )

            for k in L3A0:
                for b in range(nb):
                    l3_mms(b, [k])
            # L2's leftover applies: emitted behind ~10us of queued PE work;
            # their ARs land before the readers need them
            for ap_fn in carry2:
                ap_fn()
            for k in L3A1:
                for b in range(nb):
                    l3_mms(b, [k])

            # log_softmax tail. The raw logits are bounded (|logit| < 4:
            # BN-normalized h3 against W3 ~ N(0,1/2048)), so exp() needs no
            # max-subtraction — the f32 exp-sum stays < 1e4 — removing the
            # DVE max-reduce from the critical path. The ACT engine reloads
            # its function table on EVERY Exp<->Ln switch (1.28us), so the
            # chain is split into an Exp phase (per tile, right after its
            # stop-matmul — one hidden Exp load total) and one batched
            # Ln/writeback phase (one Ln load, the only table swap trailing
            # the final matmul).
            ssum = [None] * nb
            for b in range(nb):
                l3_mms(b, L3B)
                s0 = small.tile([P, 1], F32, name=f"s0_{b}", tag="sm")
                s1 = small.tile([P, 1], F32, name=f"s1_{b}", tag="sm")
                e0 = scratch.tile([P, 512], F32, name=f"e0_{b}", tag="sq")
                e1 = scratch.tile([P, 512], F32, name=f"e1_{b}", tag="sq")
                nc.scalar.activation(
                    e0[:, :half], ps3[b][0][:, :half], AF.Exp,
                    accum_out=s0[:],
                )
                nc.scalar.activation(
                    e1[:, :half], ps3[b][1][:, :half], AF.Exp,
                    accum_out=s1[:],
                )
                ssum[b] = small.tile([P, 1], F32, name=f"ssum_{b}", tag="sm")
                nc.vector.tensor_add(ssum[b][:], s0[:], s1[:])
            for b in range(nb):
                lse = small.tile([P, 1], F32, name=f"lse_{b}", tag="sm")
                shift = small.tile([P, 1], F32, name=f"shift_{b}", tag="sm")
                nc.scalar.activation(lse[:], ssum[b][:], AF.Ln)
                nc.vector.tensor_scalar_mul(shift[:], lse[:], -1.0)
                # writeback split across ACT and DVE so the two halves of
                # each tile shift in parallel; dedicated pool so a tile's
                # writeback never waits on an earlier tile's store DMA to
                # release a shared scratch buffer
                o0 = outp.tile([P, 512], F32, name=f"o0_{b}", tag="out")
                o1 = outp.tile([P, 512], F32, name=f"o1_{b}", tag="out")
                nc.scalar.activation(
                    o0[:, :half], ps3[b][0][:, :half], AF.Identity,
                    bias=shift[:], scale=1.0,
                )
                nc.vector.tensor_scalar_add(
                    o1[:, :half], ps3[b][1][:, :half], shift[:]
                )
                # halves on different DMA queues so the 8 stores drain in
                # parallel instead of serializing on sync
                nc.sync.dma_start(out[b * P : (b + 1) * P, :half], o0[:, :half])
                nc.gpsimd.dma_start(out[b * P : (b + 1) * P, half:C], o1[:, :half])

    nc.compile()
    return nc


def prep_inputs(inputs, b_shard: int, n_cores: int):
    """Host-side prep: shard x, transpose/cast weights, pack BN params."""
    x = np.ascontiguousarray(inputs["x"], dtype=np.float32)

    def bf(a):
        return np.ascontiguousarray(a).astype(NP_BF16)

    def f8(a):
        return np.ascontiguousarray(a).astype(NP_F8)

    def sign_f32(w):
        return np.where(w >= 0, np.float32(1.0), np.float32(-1.0))

    ks = KF8_START * P
    w0T = inputs["W0"].astype(np.float32).T  # [D_IN, H1]
    w0t = bf(w0T[:ks])
    w0t8 = f8(XDIV * w0T[ks:])
    w1t = f8(sign_f32(np.asarray(inputs["Wb1"], dtype=np.float32)).T)
    w2t = f8(sign_f32(np.asarray(inputs["Wb2"], dtype=np.float32)).T)
    w3t_aug = np.zeros(((KT3 + 1) * P, C), dtype=np.float32)
    w3t_aug[:H3] = inputs["W3"].astype(np.float32).T
    w3t_aug[H3] = inputs["b3"].astype(np.float32)
    w3t_aug = bf(w3t_aug)

    def pack(v, mt):
        return np.ascontiguousarray(
            np.asarray(v, dtype=np.float32).reshape(mt, P).T
        )

    shared = {
        "w0t": w0t,
        "w0t8": w0t8,
        "w1t": w1t,
        "w2t": w2t,
        "w3t": w3t_aug,
        "g0p": pack(inputs["g0"], MT0),
        "b0p": pack(inputs["beta0"], MT0),
        "g1p": pack(inputs["g1"], MT1),
        "b1p": pack(inputs["beta1"], MT1),
        "g2p": pack(inputs["g2"], MT2),
        "b2p": pack(inputs["beta2"], MT2),
    }
    in_maps = []
    for i in range(n_cores):
        xs = x[i * b_shard : (i + 1) * b_shard]  # [b_shard, D_IN]
        m = dict(shared)
        m["xT"] = bf(xs.T[:ks])  # bf16 head [KF8_START*P, b_shard]
        m["xT8"] = f8(xs.T[ks:] / XDIV)  # fp8 tail, scale cancels vs XDIV*W0
        in_maps.append(m)
    return in_maps


_CACHE = {}


def _get_compiled(b_shard: int, n_cores: int):
    key = (b_shard, n_cores)
    if key not in _CACHE:
        _CACHE[key] = build(b_shard, n_cores)
    return _CACHE[key]


def kernel(**inputs) -> np.ndarray:
    b_shard = B_TOTAL // N_CORES
    nc = _get_compiled(b_shard, N_CORES)
    in_maps = prep_inputs(inputs, b_shard, N_CORES)
    last_err = None
    for _attempt in range(3):
        try:
            res = run_bass_kernel_spmd(nc, in_maps, core_ids=list(range(N_CORES)))
            break
        except Exception as e:  # transient NRT device flakes recover on retry
            last_err = e
            # a wedged exec unit persists in the live PJRT backend; force a
            # backend re-init so the retry reopens (and resets) the device
            try:
                import jax
                import time
                from jax._src import xla_bridge as _xb

                jax.clear_caches()
                _xb._clear_backends()
                time.sleep(5.0)
            except Exception:
                pass
    else:
        raise last_err
    out = np.concatenate([r["out"] for r in res.results], axis=0)
    return out.astype(np.float32)


if __name__ == "__main__":
    data = np.load("/tmp/ref_data.npz")
    inputs = {k: data[k] for k in data.files if k != "expected"}
    expected = data["expected"]
    actual = kernel(**inputs)
    err = np.abs(actual - expected)
    print("max abs err:", err.max())
    print("absmax-rel:", err.max() / np.abs(expected).max())


# revision 31
# speedup vs baseline: 1.0347x; 1.0347x over previous
"""BinaryMLP (dense_mlp) Trainium2 kernel — 8-core data-parallel sync-BN.

Strategy:
  - Shard batch (4096) across 8 NeuronCores (512 rows each); replicate weights.
  - Activations live in SBUF transposed: [features -> partitions, batch -> free].
    BatchNorm batch stats are then free-axis reductions (DVE / ACT accum).
  - Matmuls: lhsT = W.T tile (stationary), rhs = xT tile (moving), fp32 PSUM
    accumulation. The PE is power-throttled to K=13/16 (1.95 GHz) in steady
    state, so each 512-row matmul issues every ~263 ns with LDWEIGHTS fully
    hidden — the kernel is instruction-count-bound on the PE.
  - Layer 0 runs a 12-k-tile bf16 head plus a 20-k-tile fp8e4 DoubleRow tail
    (x/4 vs 4*W0 — the power-of-2 scales cancel exactly in the product);
    layers 1/2 run fully fp8 DoubleRow (sign() weights are exactly +-1 in
    fp8). h1/h2 are stored offset-coded (h - 0.40625) to center the
    post-ReLU distribution; the constant per-feature shift downstream is
    absorbed exactly by the next BatchNorm's mean subtraction.
  - L0's k-loop runs the fp8 chunks FIRST: the fp8 x tail (1.3 MB) loads in
    a few us so the PE starts ~11 us in, while the bigger bf16 x head
    streams in behind it.
  - Sync-BN: per-group pipelined stats with BATCHED AllReduces (2 groups =
    8 KB per AR). Each AR pays a ~10 us ncfw latency floor on the serial CC
    engine, so fewer, earlier-triggered ARs make the last apply land ~15 us
    after a layer's last matmul instead of ~50 us. Pending applies drain
    BEFORE the next group's stats in the in-order ACT/DVE queues, so an
    in-flight AR never delays the stats that free PSUM banks for the PE.
    Each consumer layer defers the k-chunks that read the producer's last
    two groups (kb) behind ~40 us of covered pass-A work.
  - Final Linear flips layout back to [batch -> partitions, classes -> free]
    by using the h3 activation tiles as the stationary operand; log_softmax
    is a free-axis max/exp-accum/ln chain; b3 is folded in via an extra
    ones-row x b3-row contraction tile. The Exp/Ln ACT tables are re-warmed
    right after the last BN apply (whose Sqrt evicts them) so no 1.3 us
    table load lands on the softmax critical path.
"""

import os
import sys

for _p in ("/opt/trn_rl_repo",):
    if _p not in sys.path and os.path.isdir(_p):
        sys.path.insert(0, _p)

import numpy as np
import ml_dtypes

import concourse.bass as bass
import concourse.mybir as mybir
import concourse.tile as tile
from concourse import bacc
from concourse.bass_utils import run_bass_kernel_spmd

AF = mybir.ActivationFunctionType
ALU = mybir.AluOpType
F32 = mybir.dt.float32
BF16 = mybir.dt.bfloat16
F8 = mybir.dt.float8e4
AX = mybir.AxisListType
PM = mybir.MatmulPerfMode

NP_BF16 = ml_dtypes.bfloat16
NP_F8 = ml_dtypes.float8_e4m3

P = 128
N_CORES = 8
B_TOTAL = 4096
D_IN = 4096
H1, H2, H3 = 4096, 4096, 2048
C = 1000
BN_EPS = 1e-5

KT0, MT0 = D_IN // P, H1 // P  # 32, 32
KT1, MT1 = H1 // P, H2 // P  # 32, 32
KT2, MT2 = H2 // P, H3 // P  # 32, 16
KT3 = H3 // P  # 16 (+1 aug tile for the bias)
MG = 4  # out-feature tiles per PSUM group (4 banks; 2 groups in flight)
KPAIR = 4  # k-tiles per weight-slab DMA (512KB slabs amortize DMA fixed cost)
# fp8 activation offset: h1/h2 are stored as (relu(bn(h)) - OFFC) in e4m3,
# centering the post-ReLU distribution to cut quantization noise ~30%. The
# resulting constant per-feature shift downstream is absorbed by the next
# BatchNorm's mean subtraction. 13/32 is exact in both bf16 and e4m3.
OFFC = 0.40625
# Layer 0 runs k-tiles KF8_START..31 in fp8 DoubleRow: x/XDIV in e4m3 against
# XDIV*W0 in e4m3 — the power-of-2 scales cancel exactly in the product, so
# the fp8 partial sums accumulate into the same PSUM as the bf16 head with no
# correction. XDIV=4 keeps small |x| out of the e4m3 denormal floor; numpy
# simulation of the full pipeline puts absmax-rel at 1.67e-2 (< 2e-2 gate).
KF8_START = 12
XDIV = 4.0


def build(b_shard: int, n_cores: int):
    """Build + compile the SPMD program for a per-core batch shard of b_shard."""
    assert b_shard % P == 0
    nb = b_shard // P  # batch tiles for the final layer
    n_batch_global = b_shard * n_cores
    inv_n = 1.0 / float(n_batch_global)
    rg = [list(range(n_cores))]

    nc = bacc.Bacc(
        "TRN2", target_bir_lowering=False, debug=False, num_devices=n_cores
    )

    xT = nc.dram_tensor(
        "xT", [KF8_START * P, b_shard], BF16, kind="ExternalInput"
    ).ap()
    xT8 = nc.dram_tensor(
        "xT8", [(KT0 - KF8_START) * P, b_shard], F8, kind="ExternalInput"
    ).ap()
    w0t = nc.dram_tensor(
        "w0t", [KF8_START * P, H1], BF16, kind="ExternalInput"
    ).ap()
    w0t8 = nc.dram_tensor(
        "w0t8", [(KT0 - KF8_START) * P, H1], F8, kind="ExternalInput"
    ).ap()
    w1t = nc.dram_tensor("w1t", [H1, H2], F8, kind="ExternalInput").ap()
    w2t = nc.dram_tensor("w2t", [H2, H3], F8, kind="ExternalInput").ap()
    # W3.T augmented with a b3 row (row H3) + zero padding to a full k-tile.
    w3t = nc.dram_tensor("w3t", [(KT3 + 1) * P, C], BF16, kind="ExternalInput").ap()
    g0p = nc.dram_tensor("g0p", [P, MT0], F32, kind="ExternalInput").ap()
    b0p = nc.dram_tensor("b0p", [P, MT0], F32, kind="ExternalInput").ap()
    g1p = nc.dram_tensor("g1p", [P, MT1], F32, kind="ExternalInput").ap()
    b1p = nc.dram_tensor("b1p", [P, MT1], F32, kind="ExternalInput").ap()
    g2p = nc.dram_tensor("g2p", [P, MT2], F32, kind="ExternalInput").ap()
    b2p = nc.dram_tensor("b2p", [P, MT2], F32, kind="ExternalInput").ap()
    out = nc.dram_tensor("out", [b_shard, C], F32, kind="ExternalOutput").ap()

    with tile.TileContext(nc) as tc:
        with (
            tc.tile_pool(name="big", bufs=1) as big,
            tc.tile_pool(name="wpool", bufs=8) as wpool,
            tc.tile_pool(name="psum", bufs=8, space="PSUM") as psum,
            tc.tile_pool(name="scratch", bufs=4) as scratch,
            tc.tile_pool(name="outp", bufs=8) as outp,
            tc.tile_pool(name="bn", bufs=10) as bnp,
            tc.tile_pool(name="small", bufs=24) as small,
            tc.tile_pool(name="dram", bufs=1, space="DRAM") as dram,
        ):
            # ---- persistent activation buffers -------------------------------
            # xT_sb holds only the bf16 head (k-tiles 0..KF8_START-1); the
            # fp8 tail lives in xT8_sb.
            xT_sb = big.tile(
                [P, KF8_START, b_shard], BF16, name="xT_sb", tag="xT_sb"
            )
            xT8_sb = big.tile(
                [P, KT0 - KF8_START, b_shard], F8, name="xT8_sb", tag="xT8_sb"
            )
            h1_sb = big.tile([P, MT0, b_shard], F8, name="h1_sb", tag="h1_sb")
            h2_sb = big.tile([P, MT1, b_shard], F8, name="h2_sb", tag="h2_sb")
            h3_sb = big.tile([P, MT2, b_shard], BF16, name="h3_sb", tag="h3_sb")
            # one shared pre-BN scratch: layer N+1's first stats-write lands
            # only after its k-loop consumed ALL of layer N's output, i.e.
            # after every layer-N apply (the last pre readers) completed
            pre0 = big.tile([P, MT0, b_shard], BF16, name="pre0", tag="pre0")
            # dedicated stash buffer for spilled pass-A partials: carried-over
            # applies from the previous layer still read pre0's old tiles, so
            # the stash must not alias pre0. Two slots: L1 stashes its last
            # TWO groups' pass-A partials to extend the covered window ahead
            # of the cross-core join by another ~8us.
            stash_sb = big.tile(
                [P, 2 * MG, b_shard], BF16, name="stash_sb", tag="stash_sb"
            )
            ones_t = big.tile([P, b_shard], BF16, name="ones_t", tag="ones_t")
            w3_sb = big.tile([P, KT3 + 1, C], BF16, name="w3_sb", tag="w3_sb")

            # BN gamma/beta tiles (host packed to [P, MT]); DMAs for g0/b0 are
            # emitted from the x loader so the startup gpsimd queue stays
            # clear for the first fp8 x chunks.
            gb = {}

            def alloc_gb(specs):
                for nm, _ap, mt in specs:
                    gb[nm] = big.tile(
                        [P, mt], F32, name=f"{nm}_sb", tag=f"{nm}_sb"
                    )

            def emit_gb(specs):
                for nm, ap_, _mt in specs:
                    nc.gpsimd.dma_start(gb[nm][:], ap_)

            gb0_specs = (("g0", g0p, MT0), ("b0", b0p, MT0))
            alloc_gb(gb0_specs)

            # ---- L0 input loader --------------------------------------------
            # L0's k-chunks are emitted fp8-FIRST (order below): the fp8 x
            # tail chunks are small (256KB) and load on gpsimd within a few
            # us, so the PE starts ~11us in; the bf16 head chunks stream on
            # the sync queue (interleaved with the weight slabs) during the
            # ~21us of fp8-phase compute. w3 (4.25MB) and the g0/b0 BN params
            # are deferred into the bf16 phase so they never race the
            # startup-critical pieces.
            L0_ORDER = [12, 16, 20, 24, 28, 0, 4, 8]
            assert L0_ORDER[0] == KF8_START
            l0_pos = {kp: i for i, kp in enumerate(L0_ORDER)}
            xT_r = xT.rearrange("(ko p) b -> p ko b", p=P)
            xT8_r = xT8.rearrange("(ko p) b -> p ko b", p=P)
            piece_emitted = {kp: False for kp in L0_ORDER}
            gb0_emitted = [False]

            # fp8 x chunks alternate between the gpsimd and scalar queues
            # (both idle at startup; only gpsimd/sync/scalar can issue DMAs)
            # so the early transfers spread across more DMA rings in
            # parallel while the rings are still ramping up.
            x8_queues = [nc.gpsimd, nc.scalar]

            N_F8C = (KT0 - KF8_START) // KPAIR  # fp8 chunk count (first in order)

            def xT_loader(kp):
                pos = l0_pos[kp]
                # all fp8 pieces issue immediately (they are what the PE
                # consumes first; two queues drain them in parallel), bf16
                # pieces follow with a 2-chunk lookahead
                for p_ in range(max(min(pos + 2, len(L0_ORDER) - 1), N_F8C - 1) + 1):
                    c = L0_ORDER[p_]
                    if piece_emitted[c]:
                        continue
                    piece_emitted[c] = True
                    if c >= KF8_START:
                        c8 = c - KF8_START
                        if c8 == 0:
                            # the very first piece gates the first matmul:
                            # split it across both queues so two DMA rings
                            # pull it in parallel while they're still cold
                            h8 = KPAIR // 2
                            nc.gpsimd.dma_start(
                                xT8_sb[:, :h8, :], xT8_r[:, :h8, :]
                            )
                            nc.scalar.dma_start(
                                xT8_sb[:, h8:KPAIR, :], xT8_r[:, h8:KPAIR, :]
                            )
                        else:
                            eng = x8_queues[(c8 // KPAIR) % len(x8_queues)]
                            eng.dma_start(
                                xT8_sb[:, c8 : c8 + KPAIR, :],
                                xT8_r[:, c8 : c8 + KPAIR, :],
                            )
                    else:
                        nc.sync.dma_start(
                            xT_sb[:, c : c + KPAIR, :],
                            xT_r[:, c : c + KPAIR, :],
                        )
                if kp == 0 and not gb0_emitted[0]:
                    gb0_emitted[0] = True
                    emit_gb(gb0_specs)

            def mlp_layer(
                lidx, in_sb, kt, mt, w_dram, g_sb, b_sb, out_sb, pre_sb,
                ka=None, kb=(), in_loader=None, fp8=False, out_off=None,
                stash_groups=0, ar_batches=None, fp8_tail=None, lag=1,
                carry_in=None,
            ):
                """out_sb <- relu(bn(in_sb.T @ W.T)), transposed layout.

                Per-group pipelined sync-BN: every PSUM group's stats are
                all-reduced (batched with its AR partners) and applied after
                a short lag. ka/kb: KPAIR-aligned k-chunk lists. Pass A (ka)
                runs first for the leading groups and only touches input
                tiles whose producer applies completed early; pass B (kb)
                consumes the late tiles. stash_groups leading groups spill
                pass-A partials to the stash buffer to stay within the
                8-bank PSUM limit while extending the covered window.
                """
                ngroups = mt // MG
                S = stash_groups
                assert S <= 2, "stash_sb holds two groups' partials"
                stash_slot = {}
                if ka is None:
                    ka = list(range(0, kt, KPAIR))
                ka = list(ka)
                kb = list(kb)
                kstep = 2 if fp8 else 1
                k_first = ka[0]
                last_kp = kb[-1] if kb else ka[-1]
                last_fp8 = fp8 or (fp8_tail is not None and last_kp >= fp8_tail[0])
                k_last = last_kp + KPAIR - (2 if last_fp8 else 1)

                ps_tiles = {}
                # Stats AllReduces are BATCHED (2 groups = 8KB payload per
                # AR): the CC engine services collectives serially with a
                # ~10us ncfw latency floor each, so halving the AR count
                # makes the last apply land ~15us after the layer's last
                # matmul instead of ~50us.
                if ar_batches is None:
                    ar_batches = [[g] for g in range(ngroups)]
                bat_of = {}
                for bi, bat in enumerate(ar_batches):
                    for ii, g in enumerate(bat):
                        bat_of[g] = (bi, ii)
                stats_b = [
                    big.tile(
                        [P, 2 * MG * len(bat)], F32,
                        name=f"st{lidx}_{bi}", tag=f"st{lidx}_{bi}",
                    )
                    for bi, bat in enumerate(ar_batches)
                ]
                gstats_b = [
                    big.tile(
                        [P, 2 * MG * len(bat)], F32,
                        name=f"gst{lidx}_{bi}", tag=f"gst{lidx}_{bi}",
                    )
                    for bi, bat in enumerate(ar_batches)
                ]
                arin_b = [
                    dram.tile(
                        [P, 2 * MG * len(bat)], F32,
                        name=f"ari{lidx}_{bi}", tag=f"ari{lidx}_{bi}",
                    )
                    for bi, bat in enumerate(ar_batches)
                ]
                arout_b = [
                    dram.tile(
                        [P, 2 * MG * len(bat)], F32,
                        name=f"aro{lidx}_{bi}", tag=f"aro{lidx}_{bi}",
                    )
                    for bi, bat in enumerate(ar_batches)
                ]

                def emit_mms(g, kps, resume=False, stop_k=None):
                    if g not in ps_tiles:
                        ps_tiles[g] = [
                            psum.tile(
                                [P, b_shard], F32,
                                name=f"ps{lidx}_{g}_{kps[0]}_{j}", tag="ps",
                            )
                            for j in range(MG)
                        ]
                    ps = ps_tiles[g]
                    ks = stop_k if stop_k is not None else k_last
                    for kp in kps:
                        # fp8_tail: (start_k, in8_sb, w8_dram) — chunks at or
                        # past start_k run fp8 DoubleRow from the scaled-fp8
                        # copies (x/XDIV fp8 vs XDIV*W0 fp8: the scales cancel
                        # exactly in the product, so PSUM accumulation matches
                        # the bf16 part with no epilogue correction).
                        c_fp8 = fp8 or (fp8_tail is not None and kp >= fp8_tail[0])
                        if fp8_tail is not None and kp >= fp8_tail[0]:
                            k0t, c_in, c_w = fp8_tail
                            w_src = c_w[
                                (kp - k0t) * P : (kp - k0t + KPAIR) * P,
                                g * MG * P : (g + 1) * MG * P,
                            ]
                        else:
                            k0t, c_in, c_w = 0, in_sb, None
                            w_src = w_dram[
                                kp * P : (kp + KPAIR) * P,
                                g * MG * P : (g + 1) * MG * P,
                            ]
                        if in_loader is not None:
                            in_loader(kp)
                        slab = wpool.tile(
                            [P, KPAIR, MG * P], F8 if c_fp8 else BF16,
                            name=f"w{lidx}_{g}_{kp}", tag="wslab",
                        )
                        # during L0's ramp phase (DMA rings still cold) g1's
                        # first slabs issue from the scalar queue so the two
                        # interleaved groups' weight streams pull in parallel
                        seng = nc.sync
                        if in_loader is not None and g == 1 and l0_pos[kp] < 3:
                            seng = nc.scalar
                        seng.dma_start(
                            slab[:], w_src.rearrange("(kk p) c -> p kk c", p=P)
                        )
                        for kk in range(0, KPAIR, 2 if c_fp8 else 1):
                            k = kp + kk
                            for j in range(MG):
                                if c_fp8:
                                    nc.tensor.matmul(
                                        ps[j][:],
                                        slab[:, kk : kk + 2, j * P : (j + 1) * P],
                                        c_in[:, k - k0t : k - k0t + 2, :],
                                        start=(k == k_first and not resume),
                                        stop=(k == ks),
                                        perf_mode=PM.DoubleRow,
                                        skip_group_check=resume,
                                    )
                                else:
                                    nc.tensor.matmul(
                                        ps[j][:],
                                        slab[:, kk, j * P : (j + 1) * P],
                                        in_sb[:, k, :],
                                        start=(k == k_first and not resume),
                                        stop=(k == ks),
                                        skip_group_check=resume,
                                    )

                def emit_stash(g, slot):
                    # spill pass-A partials to the stash buffer (bf16) and
                    # release the PSUM banks for more covered-pass work
                    stash_slot[g] = slot
                    for j in range(MG):
                        nc.vector.tensor_scalar(
                            stash_sb[:, slot * MG + j, :], ps_tiles[g][j][:],
                            1.0, None, ALU.mult,
                        )
                    del ps_tiles[g]

                def emit_unstash(g):
                    # reload stashed partials into fresh PSUM banks; resumed
                    # matmuls accumulate on top with start=False
                    slot = stash_slot[g]
                    ps_tiles[g] = [
                        psum.tile(
                            [P, b_shard], F32, name=f"psr{lidx}_{g}_{j}", tag="ps"
                        )
                        for j in range(MG)
                    ]
                    for j in range(MG):
                        nc.scalar.activation(
                            ps_tiles[g][j][:], stash_sb[:, slot * MG + j, :],
                            AF.Copy,
                        )

                def emit_stats(g):
                    bi, ii = bat_of[g]
                    st = stats_b[bi]
                    off = 2 * MG * ii
                    for j in range(MG):
                        m = g * MG + j
                        # DVE: PSUM -> bf16 pre-BN copy, fused with the
                        # per-feature sum via accum_out (one PSUM read)
                        nc.vector.tensor_scalar(
                            pre_sb[:, m, :],
                            ps_tiles[g][j][:],
                            1.0,
                            None,
                            ALU.mult,
                            ALU.add,
                            accum_out=st[:, off + j : off + j + 1],
                        )
                        sq = scratch.tile(
                            [P, b_shard], F32, name=f"sq{lidx}_{m}", tag="sq"
                        )
                        # HW: only one PSUM read per DVE inst, so square on ACT
                        nc.scalar.activation(
                            sq[:],
                            ps_tiles[g][j][:],
                            AF.Square,
                            accum_out=st[:, off + MG + j : off + MG + j + 1],
                        )

                def emit_ar(bi):
                    nc.gpsimd.dma_start(arin_b[bi][:], stats_b[bi][:])
                    nc.gpsimd.collective_compute(
                        "AllReduce",
                        ALU.add,
                        replica_groups=rg,
                        ins=[arin_b[bi].opt()],
                        outs=[arout_b[bi].opt()],
                    )
                    nc.gpsimd.dma_start(gstats_b[bi][:], arout_b[bi][:])

                def emit_apply(g):
                    # s = gamma * rsqrt(var+eps); t = beta - mean*s, then
                    # relu(h*s + t) per tile (ACT), with the fp8 offset
                    # subtract on DVE when out_off is set.
                    bi, ii = bat_of[g]
                    gs = gstats_b[bi][:, 2 * MG * ii : 2 * MG * (ii + 1)]
                    m0 = g * MG
                    mex = bnp.tile([P, 2 * MG], F32, name=f"mex{lidx}_{g}", tag="bn2")
                    m2 = bnp.tile([P, MG], F32, name=f"m2{lidx}_{g}", tag="bn")
                    var = bnp.tile([P, MG], F32, name=f"var{lidx}_{g}", tag="bn")
                    inv = bnp.tile([P, MG], F32, name=f"inv{lidx}_{g}", tag="bn")
                    rstd = bnp.tile([P, MG], F32, name=f"rstd{lidx}_{g}", tag="bn")
                    s_sb = bnp.tile([P, MG], F32, name=f"s{lidx}_{g}", tag="bn")
                    t_sb = bnp.tile([P, MG], F32, name=f"t{lidx}_{g}", tag="bn")
                    tmp = bnp.tile([P, MG], F32, name=f"tmp{lidx}_{g}", tag="bn")
                    nc.scalar.activation(mex[:], gs[:], AF.Copy, scale=inv_n)
                    mean = mex[:, :MG]
                    ex2 = mex[:, MG:]
                    nc.vector.tensor_mul(m2[:], mean[:], mean[:])
                    nc.vector.tensor_sub(var[:], ex2[:], m2[:])
                    nc.vector.tensor_scalar_add(var[:], var[:], BN_EPS)
                    nc.vector.reciprocal(inv[:], var[:])
                    nc.scalar.activation(rstd[:], inv[:], AF.Sqrt)
                    nc.vector.tensor_mul(s_sb[:], rstd[:], g_sb[:, m0 : m0 + MG])
                    nc.vector.tensor_mul(tmp[:], mean[:], s_sb[:])
                    nc.vector.tensor_sub(t_sb[:], b_sb[:, m0 : m0 + MG], tmp[:])
                    for j in range(MG):
                        m = m0 + j
                        if out_off is None:
                            nc.scalar.activation(
                                out_sb[:, m, :],
                                pre_sb[:, m, :],
                                AF.Relu,
                                bias=t_sb[:, j : j + 1],
                                scale=s_sb[:, j : j + 1],
                            )
                        else:
                            nc.scalar.activation(
                                pre_sb[:, m, :],
                                pre_sb[:, m, :],
                                AF.Relu,
                                bias=t_sb[:, j : j + 1],
                                scale=s_sb[:, j : j + 1],
                            )
                            nc.vector.tensor_scalar(
                                out_sb[:, m, :],
                                pre_sb[:, m, :],
                                out_off,
                                None,
                                ALU.subtract,
                            )

                # Applies drain LAGGED by `lag` batches, and BEFORE the
                # just-finished group's stats: the drained applies' ARs are
                # a full batch cadence old (long complete), and putting them
                # ahead of the stats in the in-order ACT/DVE queues lets
                # them execute as soon as their AR lands instead of queueing
                # behind the next group's PSUM reads.
                pending = []

                def finish(g):
                    bi, ii = bat_of[g]
                    last_in_batch = ii == len(ar_batches[bi]) - 1
                    if last_in_batch:
                        while len(pending) > lag:
                            emit_apply(pending.pop(0))
                    emit_stats(g)
                    if last_in_batch:
                        emit_ar(bi)
                        pending.extend(ar_batches[bi])

                # ---- emission schedule ----------------------------------
                # Finishes run in TILE ORDER (g0, g1, ..., g_last) so the next
                # layer can consume low tiles first and defer only the last
                # groups' tiles (kb). The LAST S groups run pass A up front
                # (stash to stash_sb, freeing PSUM) to extend the covered
                # window; their unstashes hide under the last full groups'
                # matmul time and they resume at the very end.
                stashed = list(range(ngroups - S, ngroups))
                fulls = list(range(2, ngroups - S))
                if in_loader is not None:
                    # input layer: interleave the two open groups in 4-k
                    # sub-blocks so the PE starts on the first x chunk
                    assert S == 0
                    for kp in ka:
                        emit_mms(0, [kp])
                        emit_mms(1, [kp])
                else:
                    for si, g in enumerate(stashed):
                        # pass A closed with stop=True; the resume reloads the
                        # spilled partials and accumulates pass B on top
                        emit_mms(g, ka, stop_k=ka[-1] + KPAIR - kstep)
                        emit_stash(g, si)
                    emit_mms(0, ka)
                    emit_mms(1, ka)
                # the previous layer's leftover (AR-gated) applies are emitted
                # HERE — behind this layer's covered pass-A matmuls, but ahead
                # of its PE-critical stats ops, so a pending AR never
                # back-pressures the PE through the in-order DVE/ACT queues
                if carry_in:
                    for ap_fn in carry_in:
                        ap_fn()
                for g in (0, 1):
                    if kb:
                        emit_mms(g, kb)
                    finish(g)
                # the first stashed group's unstash hides under the last full
                # group's matmuls; later stashed groups unstash just before
                # the PREVIOUS one's resume (their copies overlap its ~8us of
                # kb matmuls), keeping peak PSUM at 8 banks throughout
                for i, g in enumerate(fulls):
                    emit_mms(g, ka + kb)
                    if i == len(fulls) - 1 and S >= 1:
                        emit_unstash(stashed[0])
                    finish(g)
                for si, g in enumerate(stashed):
                    if si + 1 < len(stashed):
                        emit_unstash(stashed[si + 1])
                    emit_mms(g, kb, resume=True)
                    finish(g)
                # leftover applies are NOT drained here: their ARs may still
                # be in flight, and draining would queue them ahead of the
                # next layer's stats. The caller passes them to the next
                # layer's carry_in (or the L3 epilogue).
                return [
                    (lambda gg=g: emit_apply(gg)) for g in pending
                ]

            # lag=2 for L0: before any cross-core join the cores are skewed
            # by up to ~50us (launch stagger varies run to run), so L0's ARs
            # can take 35-50us — a two-batch (~90us) drain margin keeps a
            # slow AR's apply from blocking later groups' stats (which free
            # PSUM banks for the PE) in the in-order ACT/DVE queues.
            carry0 = mlp_layer(
                0, xT_sb, KT0, MT0, w0t, gb["g0"], gb["b0"], h1_sb, pre0,
                ka=L0_ORDER, in_loader=xT_loader, out_off=OFFC,
                fp8_tail=(KF8_START, xT8_sb, w0t8), lag=2,
                ar_batches=[[0, 1], [2, 3], [4, 5], [6, 7]],
            )
            # ones_t (L3 bias row) is initialized here so its memsets sit
            # behind the startup-critical x-chunk DMAs on the gpsimd queue
            nc.gpsimd.memset(ones_t[:], 0.0)
            nc.gpsimd.memset(ones_t[:1, :], 1.0)
            gb12_specs = (
                ("g1", g1p, MT1),
                ("b1", b1p, MT1),
                ("g2", g2p, MT2),
                ("b2", b2p, MT2),
            )
            alloc_gb(gb12_specs)
            emit_gb(gb12_specs)
            # w3 (4.25MB) is issued HERE — on the gpsimd queue this sits
            # behind L0's gstats returns, so the transfer starts only once
            # L0's last AR lands (~250us), far from the startup-critical
            # x-chunk/slab window (issuing it at t~14us steals ring
            # bandwidth and stalls the PE's early feed) yet well before
            # L3 reads it (~430us).
            nc.gpsimd.dma_start(
                w3_sb[:], w3t.rearrange("(ko p) c -> p ko c", p=P)
            )
            # carry0=[4,5,6,7] (h1 tiles 16..31): L1's pass A may only read
            # tiles 0..15, whose applies were emitted inside L0. The kb
            # chunks are ordered ascending so the single cross-core JOIN
            # (waiting out the launch skew via L0's last AR) lands as late
            # as possible behind covered work, and is paid exactly once —
            # every later AR runs on skew-converged cores at ~10-15us.
            carry1 = mlp_layer(
                1, h1_sb, KT1, MT1, w1t, gb["g1"], gb["b1"], h2_sb, pre0,
                ka=list(range(0, 16, KPAIR)), kb=[16, 20, 24, 28], fp8=True,
                out_off=OFFC, stash_groups=1, lag=1,
                ar_batches=[[0, 1], [2, 3], [4, 5], [6, 7]],
                carry_in=carry0,
            )

            # carry2=[1,2,3] (h3 tiles 4..15): L3 phase A0 (bias + tiles
            # 0..3) is the only work whose applies were emitted inside L2.
            # Two ARs of two groups each: a finer split serializes extra
            # ~15us ARs on the CC engine past L2's end and stalls L3.
            carry2 = mlp_layer(
                2, h2_sb, KT2, MT2, w2t, gb["g2"], gb["b2"], h3_sb, pre0,
                ka=list(range(0, 20, KPAIR)), kb=[20, 24, 28], fp8=True,
                stash_groups=1, lag=1, ar_batches=[[0, 1], [2, 3]],
                carry_in=carry1,
            )

            # ---- final Linear + log_softmax ---------------------------------
            # lhsT = h3 tile slice (stationary), rhs = preloaded W3.T slab
            # (moving). Output flips to [batch -> partitions, classes -> free].
            # Phase A0: k-OUTER (all 4 batch tiles per k) over [bias, 0..7] —
            # covered work while L2's carried applies land. Phase A1 consumes
            # tiles 8..11; phase B runs per-batch-tile over the last 4 k with
            # stop, so each tile's softmax chain starts while the next tile's
            # matmuls run instead of the whole softmax serializing after the
            # last matmul.
            half = (C + 1) // 2  # 500
            L3A0 = [KT3] + list(range(0, 4))
            L3A1 = list(range(4, 12))
            L3B = list(range(12, KT3))
            ps3 = [
                [
                    psum.tile([P, 512], F32, name=f"ps3_{b}_{h}", tag="ps")
                    for h in range(2)
                ]
                for b in range(nb)
            ]

            def l3_mms(b, ks):
                for k in ks:
                    lhsT = (
                        h3_sb[:, k, b * P : (b + 1) * P]
                        if k < KT3
                        else ones_t[:, b * P : (b + 1) * P]
                    )
                    for h in range(2):
                        nc.tensor.matmul(
                            ps3[b][h][:, : half],
                            lhsT,
                            w3_sb[:, k, h * half : (h + 1) * half],
                            start=(k == KT3),
                            stop=(k == L3B[-1]),
                        )

            for k in L3A0:
                for b in range(nb):
                    l3_mms(b, [k])
            # L2's leftover applies: emitted behind ~10us of queued PE work;
            # their ARs land before the readers need them
            for ap_fn in carry2:
                ap_fn()
            for k in L3A1:
                for b in range(nb):
                    l3_mms(b, [k])

            # log_softmax tail. The raw logits are bounded (|logit| < 4:
            # BN-normalized h3 against W3 ~ N(0,1/2048)), so exp() needs no
            # max-subtraction — the f32 exp-sum stays < 1e4 — removing the
            # DVE max-reduce from the critical path. The ACT engine reloads
            # its function table on EVERY Exp<->Ln switch (1.28us), so the
            # chain is split into an Exp phase (per tile, right after its
            # stop-matmul — one hidden Exp load total) and one batched
            # Ln/writeback phase (one Ln load, the only table swap trailing
            # the final matmul).
            ssum = [None] * nb
            for b in range(nb):
                l3_mms(b, L3B)
                s0 = small.tile([P, 1], F32, name=f"s0_{b}", tag="sm")
                s1 = small.tile([P, 1], F32, name=f"s1_{b}", tag="sm")
                e0 = scratch.tile([P, 512], F32, name=f"e0_{b}", tag="sq")
                e1 = scratch.tile([P, 512], F32, name=f"e1_{b}", tag="sq")
                nc.scalar.activation(
                    e0[:, :half], ps3[b][0][:, :half], AF.Exp,
                    accum_out=s0[:],
                )
                nc.scalar.activation(
                    e1[:, :half], ps3[b][1][:, :half], AF.Exp,
                    accum_out=s1[:],
                )
                ssum[b] = small.tile([P, 1], F32, name=f"ssum_{b}", tag="sm")
                nc.vector.tensor_add(ssum[b][:], s0[:], s1[:])
            for b in range(nb):
                lse = small.tile([P, 1], F32, name=f"lse_{b}", tag="sm")
                shift = small.tile([P, 1], F32, name=f"shift_{b}", tag="sm")
                nc.scalar.activation(lse[:], ssum[b][:], AF.Ln)
                nc.vector.tensor_scalar_mul(shift[:], lse[:], -1.0)
                # writeback split across ACT and DVE so the two halves of
                # each tile shift in parallel; dedicated pool so a tile's
                # writeback never waits on an earlier tile's store DMA to
                # release a shared scratch buffer
                o0 = outp.tile([P, 512], F32, name=f"o0_{b}", tag="out")
                o1 = outp.tile([P, 512], F32, name=f"o1_{b}", tag="out")
                nc.scalar.activation(
                    o0[:, :half], ps3[b][0][:, :half], AF.Identity,
                    bias=shift[:], scale=1.0,
                )
                nc.vector.tensor_scalar_add(
                    o1[:, :half], ps3[b][1][:, :half], shift[:]
                )
                # halves on different DMA queues so the 8 stores drain in
                # parallel instead of serializing on sync
                nc.sync.dma_start(out[b * P : (b + 1) * P, :half], o0[:, :half])
                nc.gpsimd.dma_start(out[b * P : (b + 1) * P, half:C], o1[:, :half])

    nc.compile()
    return nc


def prep_inputs(inputs, b_shard: int, n_cores: int):
    """Host-side prep: shard x, transpose/cast weights, pack BN params."""
    x = np.ascontiguousarray(inputs["x"], dtype=np.float32)

    def bf(a):
        return np.ascontiguousarray(a).astype(NP_BF16)

    def f8(a):
        return np.ascontiguousarray(a).astype(NP_F8)

    def sign_f32(w):
        return np.where(w >= 0, np.float32(1.0), np.float32(-1.0))

    ks = KF8_START * P
    w0T = inputs["W0"].astype(np.float32).T  # [D_IN, H1]
    w0t = bf(w0T[:ks])
    w0t8 = f8(XDIV * w0T[ks:])
    w1t = f8(sign_f32(np.asarray(inputs["Wb1"], dtype=np.float32)).T)
    w2t = f8(sign_f32(np.asarray(inputs["Wb2"], dtype=np.float32)).T)
    w3t_aug = np.zeros(((KT3 + 1) * P, C), dtype=np.float32)
    w3t_aug[:H3] = inputs["W3"].astype(np.float32).T
    w3t_aug[H3] = inputs["b3"].astype(np.float32)
    w3t_aug = bf(w3t_aug)

    def pack(v, mt):
        return np.ascontiguousarray(
            np.asarray(v, dtype=np.float32).reshape(mt, P).T
        )

    shared = {
        "w0t": w0t,
        "w0t8": w0t8,
        "w1t": w1t,
        "w2t": w2t,
        "w3t": w3t_aug,
        "g0p": pack(inputs["g0"], MT0),
        "b0p": pack(inputs["beta0"], MT0),
        "g1p": pack(inputs["g1"], MT1),
        "b1p": pack(inputs["beta1"], MT1),
        "g2p": pack(inputs["g2"], MT2),
        "b2p": pack(inputs["beta2"], MT2),
    }
    in_maps = []
    for i in range(n_cores):
        xs = x[i * b_shard : (i + 1) * b_shard]  # [b_shard, D_IN]
        m = dict(shared)
        m["xT"] = bf(xs.T[:ks])  # bf16 head [KF8_START*P, b_shard]
        m["xT8"] = f8(xs.T[ks:] / XDIV)  # fp8 tail, scale cancels vs XDIV*W0
        in_maps.append(m)
    return in_maps


_CACHE = {}


def _get_compiled(b_shard: int, n_cores: int):
    key = (b_shard, n_cores)
    if key not in _CACHE:
        _CACHE[key] = build(b_shard, n_cores)
    return _CACHE[key]


def kernel(**inputs) -> np.ndarray:
    b_shard = B_TOTAL // N_CORES
    nc = _get_compiled(b_shard, N_CORES)
    in_maps = prep_inputs(inputs, b_shard, N_CORES)
    last_err = None
    for _attempt in range(3):
        try:
            res = run_bass_kernel_spmd(nc, in_maps, core_ids=list(range(N_CORES)))
            break
        except Exception as e:  # transient NRT device flakes recover on retry
            last_err = e
            # a wedged exec unit persists in the live PJRT backend; force a
            # backend re-init so the retry reopens (and resets) the device
            try:
                import jax
                import time
                from jax._src import xla_bridge as _xb

                jax.clear_caches()
                _xb._clear_backends()
                time.sleep(5.0)
            except Exception:
                pass
    else:
        raise last_err
    out = np.concatenate([r["out"] for r in res.results], axis=0)
    return out.astype(np.float32)


if __name__ == "__main__":
    data = np.load("/tmp/ref_data.npz")
    inputs = {k: data[k] for k in data.files if k != "expected"}
    expected = data["expected"]
    actual = kernel(**inputs)
    err = np.abs(actual - expected)
    print("max abs err:", err.max())
    print("absmax-rel:", err.max() / np.abs(expected).max())
